# revision 23
# baseline (speedup 1.0000x reference)
"""BERT self-attention (flash-style) Trainium2 Bass kernel.

Full inputs -> full output. Shards data-parallel over batch: batch element i
runs on NeuronCore i (B == 8 == n_cores), no collectives.

Host-side prep (cheap numpy): transpose x / Wqkv / out_w into the e-major
layouts the TensorE needs (lhsT/rhs must both be contraction-major), fold the
1/sqrt(d) scale into the q block of Wqkv, turn the key-padding mask into an
additive exp bias (0 / -30000) and the query mask into a 0/1 multiplier.

On-chip per core (S=1024, E=768, H=12, D=64):
  Scores tiles pack both heads of a pair: [sk=128, head 2p sq-half | head
  2p+1 sq-half]; the two 64-contraction matmuls run concurrently on PE
  row-tiles 0/64 and a single Exp activation (N=1024) covers both.
  Scores/ctx/qkT/v matmul emission is interleaved slot-by-slot so the tensor
  queue never head-of-line blocks behind ScalarE activations (ScalarE exp,
  ~110us total, is the pipeline floor). v is produced per-pair during the
  matching scores stage to balance TensorE load across stages.
  ctx accumulated per (head, sq-half) in [65, 512] psum via the ones-column
  trick (row 64 = softmax denominator); evacuated to SBUF fp32, denominator
  inverted with reciprocal_approx_fast, broadcast over 64 partitions with a
  rank-1 fp32 matmul, multiplied back on VectorE.
"""

import sys

if "/opt/trn_rl_repo" not in sys.path:
    sys.path.insert(0, "/opt/trn_rl_repo")

import numpy as np
import ml_dtypes

import concourse.bass as bass
import concourse.bacc as bacc
import concourse.tile as tile
from concourse import mybir
from concourse.bass_utils import run_bass_kernel_spmd

B, S, E, H = 8, 1024, 768, 12
D = E // H            # 64
NP = 128              # SBUF/PSUM partitions
EC = E // NP          # 6 e-chunks (contraction chunks)
SC = S // NP          # 8 sequence chunks
NPAIR = H // 2        # 6 head pairs
HALF = S // 2         # 512
BF16 = mybir.dt.bfloat16
F32 = mybir.dt.float32
F16 = mybir.dt.float16
EXP = mybir.ActivationFunctionType.Exp
MASK_NEG = -30000.0


def _drive(*gens):
    """Round-robin generators to completion (one yield-slice each per round)."""
    gens = [g for g in gens if g is not None]
    while gens:
        nxt = []
        for g in gens:
            try:
                next(g)
                nxt.append(g)
            except StopIteration:
                continue
        gens = nxt


def _drive_rot(pace, fillers):
    """Advance `pace` every round but only ONE filler per round (rotating) --
    fewer matmul-class switches in the tensor queue than plain round-robin."""
    fillers = [g for g in fillers if g is not None]
    i = 0
    while pace is not None or fillers:
        if pace is not None:
            try:
                next(pace)
            except StopIteration:
                pace = None
        if fillers:
            g = fillers[i % len(fillers)]
            try:
                next(g)
                i += 1
            except StopIteration:
                fillers.remove(g)


def _seq(*gens):
    """Chain generators sequentially (for work sharing one psum ring)."""
    for g in gens:
        yield from g


def _body(tc, xt, wqk, bqk, wot, bo, kq, out, with_bias):
    nc = tc.nc

    with tc.tile_pool(name="const", bufs=1) as const:
        # ---- persistent SBUF state. DMA order matters: xt first, then the
        # q/k column blocks needed by the first qkT emissions, then the rest;
        # out_w last. First real matmuls can start ~5us in. ----------------
        # Consolidated multi-dim-AP DMAs (per-DMA overhead is ~600ns, so
        # few big transfers beat many small ones), split across the two
        # hardware DMA queues (SP + Activation). Order: xt and the first
        # q/k pair columns first so qkT(0)/qkT(6) start ~6us in.
        W3 = 3 * E
        # critical set first, strictly ordered on the sync queue: the first
        # q/k pair columns + x. All 8 cores pull weights simultaneously at
        # startup, so DMA order == first-matmul latency.
        wqk0 = const.tile([NP, EC, 2, NP], BF16, name="wqk0")
        for qk in (0, 1):
            nc.sync.dma_start(
                out=wqk0[:, :, qk, :],
                in_=bass.AP(tensor=wqk, offset=qk * E,
                            ap=[[W3, NP], [NP * W3, EC], [1, NP]]),
            )
        xtb = const.tile([NP, EC, S], BF16, name="xtb")
        for h0 in (0, 3):
            nc.sync.dma_start(
                out=xtb[:, h0:h0 + 3, :],
                in_=bass.AP(tensor=xt, offset=h0 * NP * S,
                            ap=[[S, NP], [NP * S, 3], [1, S]]),
            )
        kq_sb = const.tile([NP, 2 * SC], F32)      # [kb bias | qm] packed
        nc.sync.dma_start(out=kq_sb, in_=kq[:, :])
        kb_sb = kq_sb[:, 0:SC]
        qm_sb = kq_sb[:, SC:2 * SC]
        # remaining q/k columns, v columns, out_w on the second queue
        wqrkr = const.tile([NP, EC, 2, 5 * NP], BF16, name="wqrkr")
        for qk in (0, 1):
            nc.scalar.dma_start(
                out=wqrkr[:, :, qk, :],
                in_=bass.AP(tensor=wqk, offset=qk * E + NP,
                            ap=[[W3, NP], [NP * W3, EC], [1, 5 * NP]]),
            )
        wvt = const.tile([NP, EC, E], BF16, name="wvt")
        nc.scalar.dma_start(
            out=wvt,
            in_=bass.AP(tensor=wqk, offset=2 * E,
                        ap=[[W3, NP], [NP * W3, EC], [1, E]]),
        )
        wotb = const.tile([NP, EC, E], BF16, name="wotb")
        nc.scalar.dma_start(
            out=wotb,
            in_=bass.AP(tensor=wot, offset=0,
                        ap=[[E, NP], [NP * E, EC], [1, E]]),
        )
        ones_bf = const.tile([NP, 64], BF16)       # lhsT for rank-1 broadcast mm
        nc.vector.memset(ones_bf, 1.0)
        wu = const.tile([NP, 256], BF16)           # PE warm-up operand
        nc.vector.memset(wu, 1.0)

        xt_k = [xtb[:, k, :] for k in range(EC)]
        wo_k = [wotb[:, k, :] for k in range(EC)]

        def q_lhsT(j, k):
            # lhsT columns for q pair j (j<NPAIR) or k pair j-NPAIR
            qk, pj = divmod(j, NPAIR) if j >= NPAIR else (0, j)
            qk = 1 if j >= NPAIR else 0
            pj = j - NPAIR if j >= NPAIR else j
            if pj == 0:
                return wqk0[:, k, qk, :]
            return wqrkr[:, k, qk, (pj - 1) * NP:pj * NP]

        def v_rhs(g, k):
            return wvt[:, k, g * 256:(g + 1) * 256]
        if with_bias:
            bq_sb = const.tile([NP, 3 * E // NP], F32)
            nc.sync.dma_start(out=bq_sb, in_=bqk.rearrange("(c p) -> p c", p=NP))
            bvcol = const.tile([NP, H], F32)       # v bias, col h = bias[2E+64h+p]
            nc.sync.dma_start(
                out=bvcol[0:64, :],
                in_=bass.AP(tensor=bqk, offset=2 * E, ap=[[1, 64], [64, H]]),
            )
            bo_bc = const.tile([NP, E], F32)       # out bias broadcast
            nc.sync.dma_start(
                out=bo_bc, in_=bass.AP(tensor=bo, offset=0, ap=[[0, NP], [1, E]])
            )
        else:
            bq_sb = bvcol = bo_bc = None

        _compute(tc, nc, with_bias, xt_k, q_lhsT, v_rhs, wo_k, kb_sb, qm_sb,
                 ones_bf, wu, out, bq_sb, bvcol, bo_bc)


def _compute(tc, nc, with_bias, xt_k, q_lhsT, v_rhs, wo_k, kb_sb, qm_sb,
             ones_bf, wu, out, bq_sb, bvcol, bo_bc):
    with tc.tile_pool(name="work", bufs=1) as work:
        # qT/kT: [128, j, s] bf16; partition = f within chunk. j=0..5 q pairs
        # (heads 2j,2j+1 at partitions 0-63 / 64-127), j=6..11 k pairs.
        qkT = work.tile([NP, H, S], BF16)
        # v (+ per-head denominator column): s-chunk m on partitions.
        # Head slot of 65 columns: [v(64), ones] -- the ones column makes the
        # ctx matmul emit the softmax denominator as psum row 64 for free.
        vsb = work.tile([NP, SC, H * 65], BF16)
        vsb_4d = vsb.rearrange("p m (h t) -> p m h t", t=65)
        nc.vector.memset(vsb_4d[:, :, :, 64:65], 1.0)
        # ctx.T: pair j -> partitions 0:64 head 2j, 64:128 head 2j+1; e-chunk j.
        ctxT = work.tile([NP, EC, S], BF16)

        with tc.tile_pool(name="norm", bufs=3) as normp, \
             tc.tile_pool(name="exps", bufs=34) as exps, \
             tc.tile_pool(name="osb", bufs=3) as outp, \
             tc.tile_pool(name="ps_sc", bufs=2, space="PSUM") as ps_sc, \
             tc.tile_pool(name="ps_qk", bufs=1, space="PSUM") as ps_qk, \
             tc.tile_pool(name="ps_ctx", bufs=2, space="PSUM") as ps_ctx:

            # psum budget: sc 2x[128,1024]=4 banks, qk 1x[128,1024]=2,
            # ctx 2x[128,512]=2 (ring shared by ctx accum + bcast tiles).

            def gen_warmup(n=20):
                # keep the PE busy through the initial DMA wait so HAM is at
                # K=8/8 when the first real matmul issues.
                pw = ps_sc.tile([NP, S], F32, tag="sc")
                for _ in range(n):
                    nc.tensor.matmul(pw[:, 0:256], lhsT=wu[:, 0:NP], rhs=wu,
                                     start=True, stop=True)
                    yield

            def gen_v2(g):
                # v columns for heads 4g..4g+3 (pairs 2g, 2g+1); 4 m-chunks
                # per [128, 1024] psum tile (m-chunk -> 256-col slice).
                for m0 in (0, 4):
                    pv = ps_qk.tile([NP, S], F32, tag="qk")
                    for m in range(m0, m0 + 4):
                        for k in range(EC):
                            nc.tensor.matmul(
                                pv[:, (m - m0) * 256:(m - m0 + 1) * 256],
                                lhsT=xt_k[k][:, m * NP:(m + 1) * NP],
                                rhs=v_rhs(g, k),
                                start=(k == 0), stop=(k == EC - 1),
                            )
                        yield
                    pv_v = pv.rearrange("p (m h d) -> p m h d", h=4, d=D)
                    nc.vector.tensor_copy(
                        out=vsb_4d[:, m0:m0 + 4, 4 * g:4 * g + 4, 0:64],
                        in_=pv_v)
                    yield

            def gen_qkT(j, ring):
                pq = ring.tile([NP, S], F32, tag="sc" if ring is ps_sc else "qk")
                for k in range(EC):
                    st, sp = (k == 0), (k == EC - 1)
                    for n in (0, 512):
                        nc.tensor.matmul(
                            pq[:, n:n + 512],
                            lhsT=q_lhsT(j, k),
                            rhs=xt_k[k][:, n:n + 512],
                            start=st, stop=sp,
                        )
                    if k % 2 == 1:
                        yield
                nc.vector.tensor_copy(out=qkT[:, j, :], in_=pq)
                if with_bias:
                    nc.vector.tensor_scalar_add(
                        out=qkT[:, j, :], in0=qkT[:, j, :],
                        scalar1=bq_sb[:, j:j + 1],
                    )
                yield

            pair_exps = {}

            def gen_scores(p, t0=0, t1=2 * SC):
                # tile (c, h): [sk chunk c, head 2p sq-half h | head 2p+1];
                # the two matmuls run concurrently on PE row-tiles 0/64.
                tiles = pair_exps.setdefault(p, {})
                for ti in range(t0, t1):
                    c, h = divmod(ti, 2)
                    if True:
                        st = ps_sc.tile([NP, S], F32, tag="sc")
                        nc.tensor.matmul(
                            st[:, 0:512],
                            lhsT=qkT[0:64, NPAIR + p, c * NP:(c + 1) * NP],
                            rhs=qkT[0:64, p, h * HALF:(h + 1) * HALF],
                            start=True, stop=True, tile_position=(0, 0),
                        )
                        nc.tensor.matmul(
                            st[:, 512:1024],
                            lhsT=qkT[64:128, NPAIR + p, c * NP:(c + 1) * NP],
                            rhs=qkT[64:128, p, h * HALF:(h + 1) * HALF],
                            start=True, stop=True, tile_position=(64, 0),
                        )
                        e = exps.tile([NP, S], BF16, tag="exp")
                        nc.scalar.activation(e, st, EXP, bias=kb_sb[:, c:c + 1])
                        tiles[(c, h)] = e
                        yield

            def gen_ctx_group(p, hi, half):
                tiles = pair_exps[p]
                head = 2 * p + hi
                pc = ps_ctx.tile([NP, 512], F32, tag="ctx")
                for c in range(SC):
                    nc.tensor.matmul(
                        pc[0:65, :],
                        lhsT=vsb[:, c, head * 65:(head + 1) * 65],
                        rhs=tiles[(c, half)][:, hi * 512:(hi + 1) * 512],
                        start=(c == 0), stop=(c == SC - 1),
                    )
                    if c % 2 == 1:
                        yield
                # evacuate psum (frees the ctx ring slot), invert the
                # denominator row, broadcast it over 64 partitions with a
                # rank-1 bf16 matmul through the same psum ring, multiply.
                cu = normp.tile([NP, 512], F32, tag="cu")
                nc.vector.tensor_copy(out=cu[0:65, :], in_=pc[0:65, :])
                rr = normp.tile([NP, 512], F32, tag="rr")
                # reciprocal_approx_fast writes nothing on HW unless the AP
                # starts at partition 0; cover 0:65 (rows 0:64 are garbage
                # reciprocals of ctx values we never read).
                with nc.allow_low_precision(reason="softmax denom recip"):
                    nc.vector.reciprocal_approx_fast(
                        out=rr[0:65, :], in_=cu[0:65, :])
                rrb = normp.tile([NP, 512], BF16, tag="rrb")
                nc.vector.tensor_copy(out=rrb[64:65, :], in_=rr[64:65, :])
                yield
                bc = ps_ctx.tile([NP, 512], F32, tag="ctx")
                nc.tensor.matmul(
                    bc[0:64, :],
                    lhsT=ones_bf[64:65, 0:64],
                    rhs=rrb[64:65, :],
                    start=True, stop=True,
                )
                n0 = half * 512
                if hi == 0:
                    dst = ctxT[0:64, p, n0:n0 + 512]
                    nc.vector.tensor_mul(out=dst, in0=cu[0:64, :], in1=bc[0:64, :])
                    if with_bias:
                        nc.vector.tensor_scalar_add(
                            out=dst, in0=dst, scalar1=bvcol[0:64, head:head + 1])
                else:
                    # DVE lanes are partition-locked; multiply at partitions
                    # 0..63 and DMA-shift to 64..127.
                    tmp = normp.tile([NP, 512], BF16, tag="sh")
                    nc.vector.tensor_mul(
                        out=tmp[0:64, :], in0=cu[0:64, :], in1=bc[0:64, :])
                    if with_bias:
                        nc.vector.tensor_scalar_add(
                            out=tmp[0:64, :], in0=tmp[0:64, :],
                            scalar1=bvcol[0:64, head:head + 1])
                    nc.sync.dma_start(
                        out=ctxT[64:128, p, n0:n0 + 512], in_=tmp[0:64, :])
                yield

            def gen_ctx(p, halves=(0, 1)):
                # interleave the two heads' groups so consecutive ctx matmuls
                # alternate psum banks (hides the same-bank accumulate
                # interlock).
                for half in halves:
                    g0 = gen_ctx_group(p, 0, half)
                    g1 = gen_ctx_group(p, 1, half)
                    for _ in zip(g0, g1):
                        yield

            def gen_outproj(ms):
                for m in ms:
                    po = ps_sc.tile([NP, S], F32, tag="sc")
                    for j in range(EC):
                        st, sp = (j == 0), (j == EC - 1)
                        nc.tensor.matmul(
                            po[:, 0:512],
                            lhsT=ctxT[:, j, m * NP:(m + 1) * NP],
                            rhs=wo_k[j][:, 0:512],
                            start=st, stop=sp,
                        )
                        nc.tensor.matmul(
                            po[:, 512:768],
                            lhsT=ctxT[:, j, m * NP:(m + 1) * NP],
                            rhs=wo_k[j][:, 512:768],
                            start=st, stop=sp,
                        )
                        yield
                    if with_bias:
                        o32 = outp.tile([NP, E], F32, tag="o32")
                        nc.vector.tensor_scalar_mul(o32, po[:, 0:768],
                                                    qm_sb[:, m:m + 1])
                        nc.vector.tensor_add(o32, o32, bo_bc)
                        osb = outp.tile([NP, E], F16, tag="osb")
                        nc.vector.tensor_copy(out=osb, in_=o32)
                    else:
                        osb = outp.tile([NP, E], F16, tag="osb")
                        nc.vector.tensor_scalar_mul(osb, po[:, 0:768],
                                                    qm_sb[:, m:m + 1])
                    nc.sync.dma_start(out=out[m * NP:(m + 1) * NP, :], in_=osb)
                    yield

            def _delayed(n, g):
                for _ in range(n):
                    yield
                yield from g

            # ---- pipelined emission --------------------------------------
            # stage 0: warm up PE through the DMA wait; qkT(0) and qkT(6) on
            # separate rings so they overlap; then scores(0) with v(0) and
            # the stage-1 qkT prefetches. Each stage pre-emits the first two
            # scores tiles of the next pair so ScalarE never idles across
            # stage boundaries.
            _drive(gen_warmup())
            _drive(gen_qkT(0, ps_qk), gen_qkT(NPAIR, ps_sc))
            _drive_rot(gen_scores(0),
                       [_seq(gen_qkT(1, ps_qk), gen_qkT(NPAIR + 1, ps_qk),
                             gen_v2(0)),
                        _delayed(14, gen_scores(1, 0, 2))])
            # steady state: scores(p) || ctx(p-1) || qkT(p+1) prefetch || v
            for p in range(1, NPAIR):
                qk_work = [gen_qkT(p + 1, ps_qk), gen_qkT(NPAIR + p + 1, ps_qk)] \
                    if p + 1 < NPAIR else []
                if p <= 2:
                    qk_work.append(gen_v2(p))
                fillers = [gen_ctx(p - 1), _seq(*qk_work)]
                if p + 1 < NPAIR:
                    fillers.append(_delayed(12, gen_scores(p + 1, 0, 2)))
                _drive_rot(gen_scores(p, 2), fillers)
            # tail: last pair's ctx; out-proj per sq-half as soon as ready
            _drive(gen_ctx(NPAIR - 1, halves=(0,)))
            _drive(gen_ctx(NPAIR - 1, halves=(1,)), gen_outproj(range(4)))
            _drive(gen_outproj(range(4, SC)))


def build_nc(with_bias=True):
    nc = bacc.Bacc()
    xt = nc.dram_tensor("xt", [E, S], BF16, kind="ExternalInput")
    wqk = nc.dram_tensor("wqkvt", [E, 3 * E], BF16, kind="ExternalInput")
    bqk = nc.dram_tensor("bqkv", [3 * E], F32, kind="ExternalInput")
    wot = nc.dram_tensor("wot", [E, E], BF16, kind="ExternalInput")
    bo = nc.dram_tensor("bo", [E], F32, kind="ExternalInput")
    kq = nc.dram_tensor("kqmask", [NP, 2 * SC], F32, kind="ExternalInput")
    out = nc.dram_tensor("out", [S, E], F16, kind="ExternalOutput")
    with tile.TileContext(nc) as tc:
        _body(tc, xt, wqk, bqk, wot, bo, kq, out, with_bias)
    nc.compile()
    return nc


def prep_in_maps(x, key_padding_mask, Wqkv_w, Wqkv_b, out_w, out_b):
    bf16 = ml_dtypes.bfloat16
    x = np.asarray(x, np.float32)
    mask = np.asarray(key_padding_mask).astype(bool)
    scale = 1.0 / np.sqrt(np.float32(D))

    wqkvT = np.asarray(Wqkv_w, np.float32).T.copy()      # (E, 3E), e-major
    wqkvT[:, :E] *= scale                                # fold 1/sqrt(d) into Wq
    bqkv = np.asarray(Wqkv_b, np.float32).copy()
    bqkv[:E] *= scale
    wotT = np.asarray(out_w, np.float32).T.copy()        # (E, E), e-major

    wqkvT = np.ascontiguousarray(wqkvT).astype(bf16)
    wotT = np.ascontiguousarray(wotT).astype(bf16)
    bo_ = np.asarray(out_b, np.float32)

    in_maps = []
    for i in range(B):
        xti = np.ascontiguousarray(x[i].T).astype(bf16)  # (E, S)
        kbias = np.where(mask[i], 0.0, MASK_NEG).astype(np.float32)
        qmask = mask[i].astype(np.float32)
        kqm = np.concatenate(
            [kbias.reshape(8, 128).T, qmask.reshape(8, 128).T], axis=1
        ).astype(np.float32)
        in_maps.append(
            {
                "xt": xti,
                "wqkvt": wqkvT,
                "bqkv": bqkv,
                "wot": wotT,
                "bo": bo_,
                "kqmask": np.ascontiguousarray(kqm),
            }
        )
    return in_maps


_NC_CACHE = {}


def _get_nc(with_bias=True):
    if with_bias not in _NC_CACHE:
        _NC_CACHE[with_bias] = build_nc(with_bias)
    return _NC_CACHE[with_bias]


def kernel(x, key_padding_mask, Wqkv_w, Wqkv_b, out_w, out_b):
    in_maps = prep_in_maps(x, key_padding_mask, Wqkv_w, Wqkv_b, out_w, out_b)
    with_bias = bool(np.any(np.asarray(Wqkv_b) != 0) or np.any(np.asarray(out_b) != 0))
    nc = _get_nc(with_bias)
    res = run_bass_kernel_spmd(nc, in_maps, core_ids=list(range(B)))
    out = np.stack([res.results[i]["out"] for i in range(B)], axis=0)
    return out.astype(np.float32)


if __name__ == "__main__":
    nc = build_nc()
    print("build ok")
